# revision 1
# baseline (speedup 1.0000x reference)
"""Trainium2 Bass kernel for nn_MultiHeadAttention_61778809586301.

Head-sharded across 8 NeuronCores: core `a` computes output row-group `a`
(which, per the reference's faithful-TF recombination, is attention head `a`
across all 8 batches, concatenated batch-major along channels, then Wo+relu).

Per-core device work (all f32):
  - projections q/k/v for all 8 batches, head slice `a` (weights host-sliced,
    inputs host-transposed to feature-major so the contraction dim lands on
    SBUF partitions)
  - causal + key-mask softmax attention, exact additive-mask semantics
  - recombine via sum_c O_c @ Wo[c-slot] with relu + query-mask epilogue

Batches are processed in pairs (c, c+4) packed into SBUF partition halves so
K=64 / M=64 matmuls run as concurrent row/col-tiled pairs on the PE array.
"""
import sys

if "/opt/trn_rl_repo" not in sys.path:
    sys.path.insert(0, "/opt/trn_rl_repo")

import numpy as np

B, S, D, H, DH = 8, 1024, 512, 8, 64
NEG = np.float32(1.0e9)
NPAIR = 4          # batch pairs (p, p+4)
NBLK = S // 128    # 8 sq blocks of 128
KO = D // 128      # 4 contraction chunks of 128

_CACHE: dict = {}
RUN_KWARGS: dict = {}   # extra kwargs for run_bass_kernel_spmd (e.g. trace)
LAST_RESULT = None      # BassKernelResults of the most recent kernel() call


def _build():
    import concourse.mybir as mybir
    import concourse.tile as tile
    from concourse import bacc
    from concourse.masks import make_identity

    f32 = mybir.dt.float32
    bf16 = mybir.dt.bfloat16
    nc = bacc.Bacc(
        "TRN2",
        target_bir_lowering=False,
        debug=False,
        enable_asserts=False,
        num_devices=H,
    )

    xt_q = nc.dram_tensor("xt_q", [D, B * S], bf16, kind="ExternalInput")
    xt_k = nc.dram_tensor("xt_k", [D, B * S], bf16, kind="ExternalInput")
    xt_v = nc.dram_tensor("xt_v", [D, B * S], bf16, kind="ExternalInput")
    wq_d = nc.dram_tensor("wq", [D, DH], bf16, kind="ExternalInput")
    wk_d = nc.dram_tensor("wk", [D, DH], bf16, kind="ExternalInput")
    wv_d = nc.dram_tensor("wv", [D, DH], bf16, kind="ExternalInput")
    wo_d = nc.dram_tensor("wo_p", [NPAIR, 128, D], bf16, kind="ExternalInput")
    madd_d = nc.dram_tensor("madd", [S, S], f32, kind="ExternalInput")
    kmb_d = nc.dram_tensor("kmb", [128, S], bf16, kind="ExternalInput")
    n1t_d = nc.dram_tensor("n1t", [128, NBLK], f32, kind="ExternalInput")
    qm_d = nc.dram_tensor("qmask", [128, NBLK], f32, kind="ExternalInput")
    out_d = nc.dram_tensor("out", [S, D], f32, kind="ExternalOutput")

    with tile.TileContext(nc) as tc:
        with (
            tc.tile_pool(name="fixed", bufs=1) as fixed,
            tc.tile_pool(name="stage", bufs=3) as stage,
            tc.tile_pool(name="proj", bufs=2) as proj,
            tc.tile_pool(name="work", bufs=3) as work,
            tc.tile_pool(name="ptp", bufs=4) as ptp,
            tc.tile_pool(name="stats", bufs=6) as stats,
            tc.tile_pool(name="psA", bufs=4, space="PSUM") as psA,
            tc.tile_pool(name="psB", bufs=2, space="PSUM") as psB,
        ):
            # ---- constants / weights ----
            ident = fixed.tile([128, 128], f32, tag="ident")
            make_identity(nc, ident[:])
            ident_bf = fixed.tile([128, 128], bf16, tag="identbf")
            nc.vector.tensor_copy(ident_bf[:], ident[:])

            wq_sb = fixed.tile([128, KO, DH], bf16, tag="wq")
            wk_sb = fixed.tile([128, KO, DH], bf16, tag="wk")
            wv_sb = fixed.tile([128, KO, DH], bf16, tag="wv")
            nc.sync.dma_start(wq_sb[:], wq_d.rearrange("(ko ki) m -> ki ko m", ki=128))
            nc.sync.dma_start(wk_sb[:], wk_d.rearrange("(ko ki) m -> ki ko m", ki=128))
            nc.sync.dma_start(wv_sb[:], wv_d.rearrange("(ko ki) m -> ki ko m", ki=128))

            wo_sb = fixed.tile([128, NPAIR, D], bf16, tag="wo")
            nc.sync.dma_start(wo_sb[:], wo_d.rearrange("p ki n -> ki p n"))

            qm_sb = fixed.tile([128, NBLK], f32, tag="qm")
            nc.sync.dma_start(qm_sb[:], qm_d[:, :])

            kmb_sb = fixed.tile([128, S], bf16, tag="kmb")
            nc.sync.dma_start(kmb_sb[:], kmb_d[:, :])
            n1t_sb = fixed.tile([128, NBLK], f32, tag="n1t")
            nc.sync.dma_start(n1t_sb[:], n1t_d[:, :])
            neg_big = fixed.tile([128, 1], f32, tag="negbig")
            nc.vector.memset(neg_big[:], -1.0e9)

            madd_sb = []
            for i in range(NBLK):
                L = 128 * (i + 1)
                t = fixed.tile([128, L], f32, tag=f"madd{i}")
                nc.sync.dma_start(t[:], madd_d[128 * i:128 * (i + 1), :L])
                madd_sb.append(t)

            # persistent attention outputs, transposed: [dh of c | dh of c+4] x S
            ot_sb = [
                fixed.tile([128, S], bf16, tag=f"ot{p}", name=f"ot{p}")
                for p in range(NPAIR)
            ]

            def emit_proj(p):
                """Projections + masked-V suffix sums + V-natural for pair p."""
                qkv_pair = []
                for name, xt, w_sb in (
                    ("q", xt_q, wq_sb), ("k", xt_k, wk_sb), ("v", xt_v, wv_sb),
                ):
                    pair_t = proj.tile([128, S], bf16, tag=f"{name}T",
                                       name=f"{name}T{p}")
                    for g in range(2):
                        c = p + 4 * g
                        st = stage.tile([128, KO, S], bf16, tag="xstage",
                                        name=f"st{p}{name}{g}")
                        nc.sync.dma_start(
                            st[:],
                            xt[:, c * S:(c + 1) * S].rearrange(
                                "(ko ki) s -> ki ko s", ki=128
                            ),
                        )
                        for hh in range(2):
                            ps = psA.tile([128, 512], f32, tag="ps",
                                          name=f"psp{p}{name}{g}{hh}")
                            for ko in range(KO):
                                nc.tensor.matmul(
                                    ps[64 * g:64 * (g + 1), :],
                                    lhsT=w_sb[:, ko, :],
                                    rhs=st[:, ko, 512 * hh:512 * (hh + 1)],
                                    start=(ko == 0),
                                    stop=(ko == KO - 1),
                                )
                            nc.vector.tensor_copy(
                                pair_t[64 * g:64 * (g + 1), 512 * hh:512 * (hh + 1)],
                                ps[64 * g:64 * (g + 1), :],
                            )
                    qkv_pair.append(pair_t)
                qT, kT, vT = qkv_pair

                # masked-V suffix sums: vks[:, i] = sum_{j>=128(i+1), km=1} v[j]
                vks = proj.tile([128, NBLK], f32, tag="vks", name=f"vks{p}")
                bsum = proj.tile([128, NBLK], f32, tag="bsum", name=f"bsum{p}")
                for b in range(NBLK):
                    ttscr = work.tile([128, 128], f32, tag="ttscr",
                                      name=f"tts{p}{b}")
                    nc.vector.tensor_tensor(
                        ttscr[:],
                        vT[:, 128 * b:128 * (b + 1)],
                        kmb_sb[:, 128 * b:128 * (b + 1)],
                        mybir.AluOpType.mult,
                    )
                    nc.vector.tensor_reduce(
                        bsum[:, b:b + 1],
                        ttscr[:],
                        axis=mybir.AxisListType.X,
                        op=mybir.AluOpType.add,
                    )
                nc.vector.memset(vks[:, NBLK - 1:NBLK], 0.0)
                for b in range(NBLK - 2, -1, -1):
                    nc.vector.tensor_tensor(
                        vks[:, b:b + 1],
                        vks[:, b + 1:b + 2],
                        bsum[:, b + 1:b + 2],
                        mybir.AluOpType.add,
                    )

                # V back to natural layout [sk, dh] per (block j, g)
                vnat = proj.tile([128, NBLK, 2, DH], bf16, tag="vnat",
                                 name=f"vnat{p}")
                for g in range(2):
                    for j in range(NBLK):
                        nc.sync.dma_start_transpose(
                            vnat[:, j, g, :],
                            vT[64 * g:64 * (g + 1), 128 * j:128 * (j + 1)],
                        )
                return qT, kT, vT, vks, vnat

            def emit_attn(p, tiles):
                qT, kT, vT, vks, vnat = tiles
                for i in range(NBLK):
                    for g in range(2):
                        L = 128 * (i + 1)
                        nch = (L + 511) // 512
                        t_sb = work.tile([128, S], f32, tag="t")
                        for n in range(nch):
                            Ln = min(512, L - 512 * n)
                            ps = psA.tile([128, 512], f32, tag="ps")
                            nc.tensor.matmul(
                                ps[:, :Ln],
                                lhsT=qT[64 * g:64 * (g + 1),
                                        128 * i:128 * (i + 1)],
                                rhs=kT[64 * g:64 * (g + 1),
                                       512 * n:512 * n + Ln],
                                start=True,
                                stop=True,
                            )
                            # t = min(sims, mclip): masked entries become the
                            # exact post-mask constants (-1e9/-2e9, matching
                            # jax's f32 absorption of "x - 1e9"), unmasked pass
                            # through (mclip=+FLT_MAX). Bit-exact tie semantics
                            # independent of engine rounding modes.
                            nc.vector.tensor_tensor(
                                t_sb[:, 512 * n:512 * n + Ln],
                                ps[:, :Ln],
                                madd_sb[i][:, 512 * n:512 * n + Ln],
                                mybir.AluOpType.min,
                            )
                        mneg = stats.tile([128, 1], f32, tag="mneg")
                        nc.vector.tensor_reduce(
                            mneg[:],
                            t_sb[:, :L],
                            axis=mybir.AxisListType.X,
                            op=mybir.AluOpType.max,
                            negate=True,
                        )
                        p_sb = work.tile([128, S], f32, tag="p")
                        ssum = stats.tile([128, 1], f32, tag="ssum")
                        nc.scalar.activation(
                            p_sb[:, :L],
                            t_sb[:, :L],
                            mybir.ActivationFunctionType.Exp,
                            bias=mneg[:],
                            scale=1.0,
                            accum_out=ssum[:],
                        )
                        # virtual tail column: weight exp(-1e9 - m) per row
                        # (0 for normal rows; 1 for fully-masked rows), with
                        # n1t tail-tie count folded into the softmax sum.
                        etail = stats.tile([128, 1], f32, tag="etail")
                        nc.scalar.activation(
                            etail[:],
                            neg_big[:],
                            mybir.ActivationFunctionType.Exp,
                            bias=mneg[:],
                            scale=1.0,
                        )
                        etn = stats.tile([128, 1], f32, tag="etn")
                        nc.vector.tensor_tensor(
                            etn[:], etail[:], n1t_sb[:, i:i + 1],
                            mybir.AluOpType.mult,
                        )
                        ssum2 = stats.tile([128, 1], f32, tag="ssum2")
                        nc.vector.tensor_tensor(
                            ssum2[:], ssum[:], etn[:], mybir.AluOpType.add,
                        )
                        rcp = stats.tile([128, 1], f32, tag="rcp")
                        nc.vector.reciprocal(rcp[:], ssum2[:])
                        ptail = stats.tile([128, 1], f32, tag="ptail")
                        nc.vector.tensor_tensor(
                            ptail[:], etail[:], rcp[:], mybir.AluOpType.mult,
                        )
                        ptrep = stats.tile([128, DH], f32, tag="ptrep")
                        nc.vector.tensor_copy(
                            ptrep[:], ptail[:, 0:1].to_broadcast((128, DH)),
                        )
                        pn_sb = work.tile([128, S], bf16, tag="pn")
                        nc.scalar.activation(
                            pn_sb[:, :L],
                            p_sb[:, :L],
                            mybir.ActivationFunctionType.Identity,
                            bias=0.0,
                            scale=rcp[:],
                        )
                        # transpose P blocks and accumulate PV into OT
                        po = psB.tile([128, 128], f32, tag="pvo")
                        for j in range(i + 1):
                            pt_sb = ptp.tile([128, 128], bf16, tag="pt")
                            nc.sync.dma_start_transpose(
                                pt_sb[:],
                                pn_sb[:, 128 * j:128 * (j + 1)],
                            )
                            nc.tensor.matmul(
                                po[64 * g:64 * (g + 1), :],
                                lhsT=vnat[:, j, g, :],
                                rhs=pt_sb[:],
                                start=(j == 0),
                                stop=(j == i),
                            )
                        # tail contribution: ot = po + ptail[sq] * vks[d]
                        gs = slice(64 * g, 64 * (g + 1))
                        btail = psB.tile([128, 128], f32, tag="pb")
                        nc.tensor.matmul(
                            btail[gs, :],
                            lhsT=ptrep[:],
                            rhs=ident[:],
                            start=True,
                            stop=True,
                        )
                        ttl = work.tile([128, 128], f32, tag="ttscr")
                        nc.vector.tensor_tensor(
                            ttl[gs, :],
                            btail[gs, :],
                            vks[gs, i:i + 1].to_broadcast((64, 128)),
                            mybir.AluOpType.mult,
                        )
                        nc.vector.tensor_tensor(
                            ot_sb[p][gs, 128 * i:128 * (i + 1)],
                            po[gs, :],
                            ttl[gs, :],
                            mybir.AluOpType.add,
                        )

            # ---- software-pipelined emission: proj one pair ahead ----
            tiles = emit_proj(0)
            nxt = emit_proj(1)
            emit_attn(0, tiles)
            tiles, nxt = nxt, emit_proj(2)
            emit_attn(1, tiles)
            tiles, nxt = nxt, emit_proj(3)
            emit_attn(2, tiles)
            emit_attn(3, nxt)

            # ---- final projection + relu + query-mask ----
            for i in range(NBLK):
                ps = psA.tile([128, 512], f32, tag="ps", name=f"psf{i}")
                for p in range(NPAIR):
                    nc.tensor.matmul(
                        ps[:],
                        lhsT=ot_sb[p][:, 128 * i:128 * (i + 1)],
                        rhs=wo_sb[:, p, :],
                        start=(p == 0),
                        stop=(p == NPAIR - 1),
                    )
                o_sb = work.tile([128, D], f32, tag="osb")
                nc.scalar.activation(
                    o_sb[:],
                    ps[:],
                    mybir.ActivationFunctionType.Relu,
                    bias=0.0,
                    scale=qm_sb[:, i:i + 1],
                )
                nc.sync.dma_start(out_d[128 * i:128 * (i + 1), :], o_sb[:])

    nc.compile()
    return nc


def _get_nc():
    if "nc" not in _CACHE:
        _CACHE["nc"] = _build()
    return _CACHE["nc"]


def _host_prep(query, key, value, query_mask, key_mask, Wq, Wk, Wv, Wo):
    """Build the 8 per-core input maps (numpy only)."""
    inv = np.float32(1.0) / np.sqrt(np.float32(D))

    import ml_dtypes

    def tfeat(x):  # (B,S,D) -> feature-major (D, B*S), contiguous bf16
        return np.ascontiguousarray(
            x.reshape(B * S, D).astype(np.float32, copy=False).T
        ).astype(ml_dtypes.bfloat16)

    xq, xk, xv = tfeat(query), tfeat(key), tfeat(value)
    kmf = key_mask.astype(np.float32)
    qmf = query_mask.astype(np.float32)
    causal = np.triu(np.full((S, S), NEG, np.float32), k=1)
    Wqf = Wq.astype(np.float32, copy=False)
    Wkf = Wk.astype(np.float32, copy=False)
    Wvf = Wv.astype(np.float32, copy=False)
    Wof = Wo.astype(np.float32, copy=False)

    wo_p = np.stack(
        [
            np.concatenate(
                [Wof[p * DH:(p + 1) * DH, :], Wof[(p + 4) * DH:(p + 5) * DH, :]],
                axis=0,
            )
            for p in range(NPAIR)
        ]
    ).astype(ml_dtypes.bfloat16)  # (4, 128, 512)

    FBIG = np.finfo(np.float32).max
    in_maps = []
    for a in range(H):
        madd_sum = (causal + NEG * (1.0 - kmf[a])[None, :]).astype(np.float32)
        # min-clip tile: exact masked values where masked, +FLT_MAX where not
        madd = np.where(madd_sum > 0, -madd_sum, FBIG).astype(np.float32)
        in_maps.append(
            {
                "xt_q": xq,
                "xt_k": xk,
                "xt_v": xv,
                "wq": np.ascontiguousarray(
                    Wqf[:, a * DH:(a + 1) * DH] * inv
                ).astype(ml_dtypes.bfloat16),
                "wk": np.ascontiguousarray(
                    Wkf[:, a * DH:(a + 1) * DH]
                ).astype(ml_dtypes.bfloat16),
                "wv": np.ascontiguousarray(
                    Wvf[:, a * DH:(a + 1) * DH]
                ).astype(ml_dtypes.bfloat16),
                "wo_p": wo_p,
                "madd": madd,
                "kmb": np.ascontiguousarray(
                    np.broadcast_to(kmf[a][None, :], (128, S))
                ).astype(ml_dtypes.bfloat16),
                "n1t": np.ascontiguousarray(
                    np.broadcast_to(
                        np.array(
                            [kmf[a, 128 * (i + 1):].sum() for i in range(NBLK)],
                            np.float32,
                        )[None, :],
                        (128, NBLK),
                    )
                ),
                "qmask": np.ascontiguousarray(
                    qmf[a].reshape(NBLK, 128).T
                ),  # [p, blk] = qm[a, 128*blk + p]
                "out": None,  # placeholder removed below
            }
        )
        del in_maps[-1]["out"]
    return in_maps


def kernel(**inputs) -> np.ndarray:
    from concourse.bass_utils import run_bass_kernel_spmd

    nc = _get_nc()
    in_maps = _host_prep(
        np.asarray(inputs["query"]),
        np.asarray(inputs["key"]),
        np.asarray(inputs["value"]),
        np.asarray(inputs["query_mask"]),
        np.asarray(inputs["key_mask"]),
        np.asarray(inputs["Wq"]),
        np.asarray(inputs["Wk"]),
        np.asarray(inputs["Wv"]),
        np.asarray(inputs["Wo"]),
    )
    res = run_bass_kernel_spmd(nc, in_maps, core_ids=list(range(H)), **RUN_KWARGS)
    global LAST_RESULT
    LAST_RESULT = res
    return np.stack([res.results[a]["out"] for a in range(H)])



# revision 16
# speedup vs baseline: 2.5635x; 2.5635x over previous
"""Trainium2 Bass kernel for nn_MultiHeadAttention_61778809586301.

Head-sharded across 8 NeuronCores: core `a` computes output row-group `a`
(which, per the reference's faithful-TF recombination, is attention head `a`
across all 8 batches, concatenated batch-major along channels, then Wo+relu).

v2 redesign vs v1:
  - scores computed TRANSPOSED (S^T = K^T blocks vs Q panels) so the P^T
    needed by the PV matmul comes straight out of exp() — zero DMA
    transposes of P (v1 spent 400+us in DMA_TRANSPOSE).
  - no max-subtraction softmax: scores are ~N(0, 1/8) so exp() is safe;
    causal-masked entries are min-clipped to -1e9 (exp -> 0).  Key-mask is
    folded into a zeroed-V (V*km) plus an extra km column, so the PV matmul
    emits [O_unnorm | rowsum] in one accumulation; normalization happens
    once on the small O^T (64 x S) instead of on P (S x S).
  - fully-masked-row semantics (the faithful-TF uniform-tie case) handled
    by host-precomputed correction columns added to O^T, plus a flag row
    seeded into the sum via a rank-1 init matmul.
  - V natural layout obtained with PE (tensor-engine) transposes.
"""
import sys

if "/opt/trn_rl_repo" not in sys.path:
    sys.path.insert(0, "/opt/trn_rl_repo")

import numpy as np

B, S, D, H, DH = 8, 1024, 512, 8, 64
NEG = np.float32(1.0e9)
FBIG = np.float32(3.0e38)
NPAIR = 4          # batch pairs (p, p+4)
NBLK = S // 128    # 8 sk/sq blocks of 128
KO = D // 128      # 4 contraction chunks of 128

_CACHE: dict = {}
RUN_KWARGS: dict = {}   # extra kwargs for run_bass_kernel_spmd (e.g. trace)
LAST_RESULT = None      # BassKernelResults of the most recent kernel() call


def _build():
    import concourse.mybir as mybir
    import concourse.tile as tile
    from concourse import bacc
    from concourse.masks import make_identity

    f32 = mybir.dt.float32
    bf16 = mybir.dt.bfloat16
    nc = bacc.Bacc(
        "TRN2",
        target_bir_lowering=False,
        debug=False,
        enable_asserts=False,
        num_devices=H,
    )

    xt_q = nc.dram_tensor("xt_q", [D, B * S], bf16, kind="ExternalInput")
    xt_k = nc.dram_tensor("xt_k", [D, B * S], bf16, kind="ExternalInput")
    xt_v = nc.dram_tensor("xt_v", [D, B * S], bf16, kind="ExternalInput")
    wq_d = nc.dram_tensor("wq", [D, DH], bf16, kind="ExternalInput")
    wk_d = nc.dram_tensor("wk", [D, DH], bf16, kind="ExternalInput")
    wv_d = nc.dram_tensor("wv", [D, DH], bf16, kind="ExternalInput")
    wo_d = nc.dram_tensor("wo_p", [NPAIR, 128, D], bf16, kind="ExternalInput")
    kmb_d = nc.dram_tensor("kmb", [128, S], bf16, kind="ExternalInput")
    kmt_d = nc.dram_tensor("kmt", [128, NBLK], bf16, kind="ExternalInput")
    tri_d = nc.dram_tensor("tri01", [128, 128], bf16, kind="ExternalInput")
    flg_d = nc.dram_tensor("flg", [1, S], bf16, kind="ExternalInput")
    corr_d = nc.dram_tensor("corrT", [128, NPAIR, 128], f32,
                            kind="ExternalInput")
    qm_d = nc.dram_tensor("qmask", [128, NBLK], f32, kind="ExternalInput")
    out_d = nc.dram_tensor("out", [S, D], f32, kind="ExternalOutput")

    with tile.TileContext(nc) as tc:
        with (
            tc.tile_pool(name="fixed", bufs=1) as fixed,
            tc.tile_pool(name="stage", bufs=3) as stage,
            tc.tile_pool(name="proj", bufs=2) as proj,
            tc.tile_pool(name="ptp", bufs=3) as ptp,
            tc.tile_pool(name="rowp", bufs=2) as rowp,
            tc.tile_pool(name="outp", bufs=3) as outp,
            tc.tile_pool(name="psS", bufs=3, space="PSUM") as psS,
            tc.tile_pool(name="psOT", bufs=2, space="PSUM") as psOT,
            tc.tile_pool(name="psT", bufs=1, space="PSUM") as psT,
        ):
            # ---- constants / weights ----
            ident = fixed.tile([128, 128], f32, tag="ident")
            make_identity(nc, ident[:])
            ident_bf = fixed.tile([128, 128], bf16, tag="identbf")
            nc.gpsimd.tensor_copy(ident_bf[:], ident[:])

            wq_sb = fixed.tile([128, KO, DH], bf16, tag="wq")
            wk_sb = fixed.tile([128, KO, DH], bf16, tag="wk")
            wv_sb = fixed.tile([128, KO, DH], bf16, tag="wv")
            nc.sync.dma_start(wq_sb[:], wq_d.rearrange("(ko ki) m -> ki ko m", ki=128))
            nc.sync.dma_start(wk_sb[:], wk_d.rearrange("(ko ki) m -> ki ko m", ki=128))
            nc.sync.dma_start(wv_sb[:], wv_d.rearrange("(ko ki) m -> ki ko m", ki=128))

            wo_sb = fixed.tile([128, NPAIR, D], bf16, tag="wo")
            nc.sync.dma_start(wo_sb[:], wo_d.rearrange("p ki n -> ki p n"))

            qm_sb = fixed.tile([128, NBLK], f32, tag="qm")
            nc.sync.dma_start(qm_sb[:], qm_d[:, :])

            kmb_sb = fixed.tile([128, S], bf16, tag="kmb")
            nc.sync.dma_start(kmb_sb[:], kmb_d[:, :])
            kmt_sb = fixed.tile([128, NBLK], bf16, tag="kmt")
            nc.sync.dma_start(kmt_sb[:], kmt_d[:, :])
            tri_sb = fixed.tile([128, 128], bf16, tag="tri")
            nc.sync.dma_start(tri_sb[:], tri_d[:, :])
            flg_sb = fixed.tile([1, S], bf16, tag="flg")
            nc.sync.dma_start(flg_sb[:], flg_d[:, :])
            corr_sb = fixed.tile([128, NPAIR, 128], f32, tag="corr")
            nc.sync.dma_start(corr_sb[:], corr_d[:, :, :])

            # unit column for the flag-row init (row 64 = 1, others 0) and
            # the ones row for the rcp broadcast
            unit64 = fixed.tile([1, DH + 1], bf16, tag="unit64")
            nc.vector.memset(unit64[:], 0.0)
            nc.vector.memset(unit64[0:1, DH:DH + 1], 1.0)
            ones64 = fixed.tile([1, DH], bf16, tag="ones64")
            nc.vector.memset(ones64[:], 1.0)

            # persistent attention outputs, transposed: [dh of c | dh of c+4] x S
            ot_sb = [
                fixed.tile([128, S], bf16, tag=f"ot{p}", name=f"ot{p}")
                for p in range(NPAIR)
            ]

            def emit_proj(p):
                """q/k/v projections for pair p; V masked+transposed+augmented."""
                qkv = []
                sts = {}
                for name, xt in (("q", xt_q), ("k", xt_k), ("v", xt_v)):
                    pair_t = proj.tile([128, S], bf16, tag=f"{name}T",
                                       name=f"{name}T{p}")
                    for g in range(2):
                        c = p + 4 * g
                        st = stage.tile([128, KO, S], bf16, tag="xstage",
                                        bufs=4, name=f"st{p}{name}{g}")
                        nc.sync.dma_start(
                            st[:],
                            xt[:, c * S:(c + 1) * S].rearrange(
                                "(ko ki) s -> ki ko s", ki=128
                            ),
                        )
                        sts[(name, g)] = st
                    w_sb = {"q": wq_sb, "k": wk_sb, "v": wv_sb}[name]
                    for hh in range(2):
                        ps = psS.tile([128, 512], f32, tag="ps",
                                      name=f"psp{p}{name}{hh}")
                        for ko in range(KO):
                            for g in range(2):
                                nc.tensor.matmul(
                                    ps[64 * g:64 * (g + 1), :],
                                    lhsT=w_sb[:, ko, :],
                                    rhs=sts[(name, g)][:, ko,
                                                       512 * hh:512 * (hh + 1)],
                                    start=(ko == 0),
                                    stop=(ko == KO - 1),
                                )
                        if name == "v":
                            # fold the key-mask into V while copying out
                            nc.vector.tensor_tensor(
                                pair_t[:, 512 * hh:512 * (hh + 1)],
                                ps[:, :],
                                kmb_sb[:, 512 * hh:512 * (hh + 1)],
                                mybir.AluOpType.mult,
                            )
                        else:
                            nc.scalar.copy(
                                pair_t[:, 512 * hh:512 * (hh + 1)],
                                ps[:, :],
                            )
                    qkv.append(pair_t)
                qT, kT, vm = qkv

                # V back to natural layout via PE transpose, both halves at
                # once: vnat[:, j, g, 0:64] = V_masked block j for batch
                # (p + 4g); vnat[:, j, g, 64] = km column for block j.
                vnat = proj.tile([128, NBLK, 2, DH + 1], bf16, tag="vnat",
                                 name=f"vnat{p}")
                for j in range(NBLK):
                    pst = psT.tile([128, 128], bf16, tag="pst",
                                   name=f"pst{p}{j}")
                    nc.tensor.transpose(
                        pst[:], vm[:, 128 * j:128 * (j + 1)], ident_bf[:]
                    )
                    nc.vector.tensor_copy(vnat[:, j, :, 0:DH], pst[:])
                nc.gpsimd.tensor_copy(
                    vnat[:, :, :, DH],
                    kmt_sb[:, :, None].to_broadcast((128, NBLK, 2)),
                )
                return qT, kT, vnat

            def emit_attn(p, tiles):
                qT, kT, vnat = tiles
                for g in range(2):
                    gs = slice(64 * g, 64 * (g + 1))
                    po = psOT.tile([DH + 1, S], f32, tag="psot",
                                   name=f"po{p}{g}")
                    # seed: rows 0..63 = 0, row 64 = flag-row indicator
                    for n in range(2):
                        cols = slice(512 * n, 512 * (n + 1))
                        nc.tensor.matmul(
                            po[:, cols],
                            lhsT=unit64[:, :],
                            rhs=flg_sb[:, cols],
                            start=True,
                            stop=False,
                            skip_group_check=True,
                        )
                    for j in range(NBLK):
                        w = S - 128 * j      # panel width (sq = 128j .. S)
                        # scores^T panel j, chunked to <=512-col psum tiles
                        chunks = []
                        if j < 4:
                            chunks = [(0, 512 - 128 * j), (512 - 128 * j, w)]
                        else:
                            chunks = [(0, w)]
                        pt = ptp.tile([128, w], bf16, tag="pt",
                                      name=f"pt{p}{g}{j}")
                        for (lo, hi) in chunks:
                            ss = psS.tile([128, 512], f32, tag="ps",
                                          name=f"ss{p}{g}{j}{lo}")
                            nc.tensor.matmul(
                                ss[:, 0:hi - lo],
                                lhsT=kT[gs, 128 * j:128 * (j + 1)],
                                rhs=qT[gs, 128 * j + lo:128 * j + hi],
                                start=True,
                                stop=True,
                            )
                            nc.scalar.activation(
                                pt[:, lo:hi],
                                ss[:, 0:hi - lo],
                                mybir.ActivationFunctionType.Exp,
                                bias=0.0,
                                scale=1.0,
                            )
                            if lo == 0:
                                # causal zeroing of the diagonal block
                                # (sk > sq entries -> 0 after exp)
                                nc.vector.tensor_tensor(
                                    pt[:, 0:128],
                                    pt[:, 0:128],
                                    tri_sb[:],
                                    mybir.AluOpType.mult,
                                )
                        # PV accumulate: po[:, sq] += vnat_j^T @ pt
                        for (lo, hi) in chunks:
                            # bank 0 (cols 0:512) last written at j=3 chunk 0;
                            # bank 1 (cols 512:1024) last written at j=7
                            last_bank = (j == 3 and lo == 0) or (j == 7)
                            nc.tensor.matmul(
                                po[:, 128 * j + lo:128 * j + hi],
                                lhsT=vnat[:, j, g, :],
                                rhs=pt[:, lo:hi],
                                start=False,
                                stop=bool(last_bank),
                                skip_group_check=True,
                            )
                    # normalization: rcp of the sum row, broadcast via PE,
                    # multiply on DVE into the persistent O^T tile.
                    rcp = rowp.tile([1, S], bf16, tag="rcp", name=f"rcp{p}{g}")
                    with nc.allow_low_precision("bf16 softmax sums"):
                        nc.vector.reciprocal(rcp[:], po[DH:DH + 1, :])
                    for n in range(2):
                        cols = slice(512 * n, 512 * (n + 1))
                        bc = psS.tile([128, 512], f32, tag="ps",
                                      name=f"bc{p}{g}{n}")
                        nc.tensor.matmul(
                            bc[0:DH, :],
                            lhsT=ones64[:, :],
                            rhs=rcp[:, cols],
                            start=True,
                            stop=True,
                        )
                        bcs = rowp.tile([DH, 512], f32, tag="bcs",
                                        name=f"bcs{p}{g}{n}")
                        nc.vector.tensor_copy(bcs[:], bc[0:DH, :])
                        nc.vector.tensor_tensor(
                            ot_sb[p][gs, cols],
                            po[0:DH, cols],
                            bcs[:],
                            mybir.AluOpType.mult,
                        )
                    # host-computed fully-masked-row corrections (block 0)
                    nc.vector.tensor_tensor(
                        ot_sb[p][gs, 0:128],
                        ot_sb[p][gs, 0:128],
                        corr_sb[gs, p, :],
                        mybir.AluOpType.add,
                    )

            # ---- software-pipelined emission: proj one pair ahead ----
            tiles = emit_proj(0)
            nxt = emit_proj(1)
            emit_attn(0, tiles)
            tiles, nxt = nxt, emit_proj(2)
            emit_attn(1, tiles)
            tiles, nxt = nxt, emit_proj(3)
            emit_attn(2, tiles)
            emit_attn(3, nxt)

            # ---- final projection + relu + query-mask ----
            for i in range(NBLK):
                ps = psS.tile([128, 512], f32, tag="ps", name=f"psf{i}")
                for p in range(NPAIR):
                    nc.tensor.matmul(
                        ps[:],
                        lhsT=ot_sb[p][:, 128 * i:128 * (i + 1)],
                        rhs=wo_sb[:, p, :],
                        start=(p == 0),
                        stop=(p == NPAIR - 1),
                    )
                o_sb = outp.tile([128, D], f32, tag="osb", name=f"osb{i}")
                nc.scalar.activation(
                    o_sb[:],
                    ps[:],
                    mybir.ActivationFunctionType.Relu,
                    bias=0.0,
                    scale=qm_sb[:, i:i + 1],
                )
                nc.sync.dma_start(out_d[128 * i:128 * (i + 1), :], o_sb[:])

    nc.compile()
    return nc


def _get_nc():
    if "nc" not in _CACHE:
        _CACHE["nc"] = _build()
    return _CACHE["nc"]


def _host_prep(query, key, value, query_mask, key_mask, Wq, Wk, Wv, Wo):
    """Build the 8 per-core input maps (numpy only)."""
    inv = np.float32(1.0) / np.sqrt(np.float32(D))

    import ml_dtypes

    def tfeat(x):  # (B,S,D) -> feature-major (D, B*S), contiguous bf16
        return np.ascontiguousarray(
            x.reshape(B * S, D).astype(np.float32, copy=False).T
        ).astype(ml_dtypes.bfloat16)

    xq, xk, xv = tfeat(query), tfeat(key), tfeat(value)
    kmf = key_mask.astype(np.float32)
    qmf = query_mask.astype(np.float32)
    Wqf = Wq.astype(np.float32, copy=False)
    Wkf = Wk.astype(np.float32, copy=False)
    Wvf = Wv.astype(np.float32, copy=False)
    Wof = Wo.astype(np.float32, copy=False)

    wo_p = np.stack(
        [
            np.concatenate(
                [Wof[p * DH:(p + 1) * DH, :], Wof[(p + 4) * DH:(p + 5) * DH, :]],
                axis=0,
            )
            for p in range(NPAIR)
        ]
    ).astype(ml_dtypes.bfloat16)  # (4, 128, 512)

    # causal 0/1 tile for the transposed diagonal block:
    # visible where sk(row) <= sq(col)
    r = np.arange(128)
    tri01 = (r[:, None] <= r[None, :]).astype(ml_dtypes.bfloat16)

    in_maps = []
    for a in range(H):
        km = kmf[a]
        # flag rows: every visible key masked -> faithful-TF uniform tie case
        cs = np.cumsum(km)
        flg = (cs == 0).astype(np.float32)           # (S,)
        corrT = np.zeros((128, NPAIR, 128), np.float32)
        flag_rows = np.nonzero(flg)[0]
        if flag_rows.size:
            assert flag_rows.max() < 128, "flag rows beyond block 0"
            wv_a = Wvf[:, a * DH:(a + 1) * DH]       # (512, 64)
            tail_cnt = km.sum()                      # km[0:sq+1] is all 0
            for p in range(NPAIR):
                for g in range(2):
                    c = p + 4 * g
                    vfull = value[c].astype(np.float32)      # (S, 512)
                    mtot = (km[:, None] * vfull).sum(axis=0)  # (512,)
                    pre = np.zeros(512, np.float32)
                    for sq in flag_rows:
                        pre = vfull[:sq + 1].sum(axis=0)
                        count = (sq + 1) + tail_cnt
                        corrT[64 * g:64 * (g + 1), p, sq] = (
                            (pre + mtot) @ wv_a
                        ) / np.float32(count)
        kmt = km.reshape(NBLK, 128).T                # [r, j] = km[128j + r]
        in_maps.append(
            {
                "xt_q": xq,
                "xt_k": xk,
                "xt_v": xv,
                "wq": np.ascontiguousarray(
                    Wqf[:, a * DH:(a + 1) * DH] * inv
                ).astype(ml_dtypes.bfloat16),
                "wk": np.ascontiguousarray(
                    Wkf[:, a * DH:(a + 1) * DH]
                ).astype(ml_dtypes.bfloat16),
                "wv": np.ascontiguousarray(
                    Wvf[:, a * DH:(a + 1) * DH]
                ).astype(ml_dtypes.bfloat16),
                "wo_p": wo_p,
                "kmb": np.ascontiguousarray(
                    np.broadcast_to(km[None, :], (128, S))
                ).astype(ml_dtypes.bfloat16),
                "kmt": np.ascontiguousarray(kmt).astype(ml_dtypes.bfloat16),
                "tri01": tri01,
                "flg": np.ascontiguousarray(flg[None, :]).astype(
                    ml_dtypes.bfloat16
                ),
                "corrT": corrT,
                "qmask": np.ascontiguousarray(
                    qmf[a].reshape(NBLK, 128).T
                ),  # [p, blk] = qm[a, 128*blk + p]
            }
        )
    return in_maps


def kernel(**inputs) -> np.ndarray:
    from concourse.bass_utils import run_bass_kernel_spmd

    nc = _get_nc()
    in_maps = _host_prep(
        np.asarray(inputs["query"]),
        np.asarray(inputs["key"]),
        np.asarray(inputs["value"]),
        np.asarray(inputs["query_mask"]),
        np.asarray(inputs["key_mask"]),
        np.asarray(inputs["Wq"]),
        np.asarray(inputs["Wk"]),
        np.asarray(inputs["Wv"]),
        np.asarray(inputs["Wo"]),
    )
    res = run_bass_kernel_spmd(nc, in_maps, core_ids=list(range(H)), **RUN_KWARGS)
    global LAST_RESULT
    LAST_RESULT = res
    return np.stack([res.results[a]["out"] for a in range(H)])


# revision 22
# speedup vs baseline: 2.7374x; 1.0678x over previous
"""Trainium2 Bass kernel for nn_MultiHeadAttention_61778809586301.

Head-sharded across 8 NeuronCores: core `a` computes output row-group `a`
(which, per the reference's faithful-TF recombination, is attention head `a`
across all 8 batches, concatenated batch-major along channels, then Wo+relu).

v2 redesign vs v1:
  - scores computed TRANSPOSED (S^T = K^T blocks vs Q panels) so the P^T
    needed by the PV matmul comes straight out of exp() — zero DMA
    transposes of P (v1 spent 400+us in DMA_TRANSPOSE).
  - no max-subtraction softmax: scores are ~N(0, 1/8) so exp() is safe;
    causal-masked entries are min-clipped to -1e9 (exp -> 0).  Key-mask is
    folded into a zeroed-V (V*km) plus an extra km column, so the PV matmul
    emits [O_unnorm | rowsum] in one accumulation; normalization happens
    once on the small O^T (64 x S) instead of on P (S x S).
  - fully-masked-row semantics (the faithful-TF uniform-tie case) handled
    by host-precomputed correction columns added to O^T, plus a flag row
    seeded into the sum via a rank-1 init matmul.
  - V natural layout obtained with PE (tensor-engine) transposes.
"""
import sys

if "/opt/trn_rl_repo" not in sys.path:
    sys.path.insert(0, "/opt/trn_rl_repo")

import numpy as np

B, S, D, H, DH = 8, 1024, 512, 8, 64
NEG = np.float32(1.0e9)
FBIG = np.float32(3.0e38)
NPAIR = 4          # batch pairs (p, p+4)
NBLK = S // 128    # 8 sk/sq blocks of 128
KO = D // 128      # 4 contraction chunks of 128

_CACHE: dict = {}
RUN_KWARGS: dict = {}   # extra kwargs for run_bass_kernel_spmd (e.g. trace)
LAST_RESULT = None      # BassKernelResults of the most recent kernel() call


def _build():
    import concourse.mybir as mybir
    import concourse.tile as tile
    from concourse import bacc
    from concourse.masks import make_identity

    f32 = mybir.dt.float32
    bf16 = mybir.dt.bfloat16
    nc = bacc.Bacc(
        "TRN2",
        target_bir_lowering=False,
        debug=False,
        enable_asserts=False,
        num_devices=H,
    )

    xt_q = nc.dram_tensor("xt_q", [D, B * S], bf16, kind="ExternalInput")
    xt_k = nc.dram_tensor("xt_k", [D, B * S], bf16, kind="ExternalInput")
    xt_v = nc.dram_tensor("xt_v", [D, B * S], bf16, kind="ExternalInput")
    wq_d = nc.dram_tensor("wq", [D, DH], bf16, kind="ExternalInput")
    wk_d = nc.dram_tensor("wk", [D, DH], bf16, kind="ExternalInput")
    wv_d = nc.dram_tensor("wv", [D, DH], bf16, kind="ExternalInput")
    wo_d = nc.dram_tensor("wo_p", [NPAIR, 128, D], bf16, kind="ExternalInput")
    kmb_d = nc.dram_tensor("kmb", [128, S], bf16, kind="ExternalInput")
    kmt_d = nc.dram_tensor("kmt", [128, NBLK], bf16, kind="ExternalInput")
    tri_d = nc.dram_tensor("tri01", [128, 128], bf16, kind="ExternalInput")
    flg_d = nc.dram_tensor("flg", [1, S], bf16, kind="ExternalInput")
    corr_d = nc.dram_tensor("corrT", [128, NPAIR, 128], f32,
                            kind="ExternalInput")
    qm_d = nc.dram_tensor("qmask", [128, NBLK], f32, kind="ExternalInput")
    out_d = nc.dram_tensor("out", [S, D], f32, kind="ExternalOutput")

    with tile.TileContext(nc) as tc:
        with (
            tc.tile_pool(name="fixed", bufs=1) as fixed,
            tc.tile_pool(name="stage", bufs=3) as stage,
            tc.tile_pool(name="proj", bufs=2) as proj,
            tc.tile_pool(name="ptp", bufs=3) as ptp,
            tc.tile_pool(name="rowp", bufs=2) as rowp,
            tc.tile_pool(name="outp", bufs=3) as outp,
            tc.tile_pool(name="psS", bufs=3, space="PSUM") as psS,
            tc.tile_pool(name="psOT", bufs=2, space="PSUM") as psOT,
            tc.tile_pool(name="psT", bufs=1, space="PSUM") as psT,
        ):
            # ---- constants / weights ----
            ident = fixed.tile([128, 128], f32, tag="ident")
            make_identity(nc, ident[:])
            ident_bf = fixed.tile([128, 128], bf16, tag="identbf")
            nc.gpsimd.tensor_copy(ident_bf[:], ident[:])

            wq_sb = fixed.tile([128, KO, DH], bf16, tag="wq")
            wk_sb = fixed.tile([128, KO, DH], bf16, tag="wk")
            wv_sb = fixed.tile([128, KO, DH], bf16, tag="wv")
            nc.sync.dma_start(wq_sb[:], wq_d.rearrange("(ko ki) m -> ki ko m", ki=128))
            nc.sync.dma_start(wk_sb[:], wk_d.rearrange("(ko ki) m -> ki ko m", ki=128))
            nc.sync.dma_start(wv_sb[:], wv_d.rearrange("(ko ki) m -> ki ko m", ki=128))

            wo_sb = fixed.tile([128, NPAIR, D], bf16, tag="wo")
            nc.sync.dma_start(wo_sb[:], wo_d.rearrange("p ki n -> ki p n"))

            qm_sb = fixed.tile([128, NBLK], f32, tag="qm")
            nc.sync.dma_start(qm_sb[:], qm_d[:, :])

            kmb_sb = fixed.tile([128, S], bf16, tag="kmb")
            nc.sync.dma_start(kmb_sb[:], kmb_d[:, :])
            kmt_sb = fixed.tile([128, NBLK], bf16, tag="kmt")
            nc.sync.dma_start(kmt_sb[:], kmt_d[:, :])
            tri_sb = fixed.tile([128, 128], bf16, tag="tri")
            nc.sync.dma_start(tri_sb[:], tri_d[:, :])
            flg_sb = fixed.tile([1, S], bf16, tag="flg")
            nc.sync.dma_start(flg_sb[:], flg_d[:, :])
            corr_sb = fixed.tile([128, NPAIR, 128], f32, tag="corr")
            nc.sync.dma_start(corr_sb[:], corr_d[:, :, :])

            # unit column for the flag-row init (row 64 = 1, others 0) and
            # the ones row for the rcp broadcast
            unit64 = fixed.tile([1, DH + 1], bf16, tag="unit64")
            nc.vector.memset(unit64[:], 0.0)
            nc.vector.memset(unit64[0:1, DH:DH + 1], 1.0)
            f32r = mybir.dt.float32r
            ones64f = fixed.tile([1, DH], f32, tag="ones64f")
            nc.vector.memset(ones64f[:], 1.0)
            ones64 = fixed.tile([1, DH], f32r, tag="ones64")
            nc.scalar.copy(ones64[:], ones64f[:])

            # persistent attention outputs, transposed: [dh of c | dh of c+4] x S
            ot_sb = [
                fixed.tile([128, S], bf16, tag=f"ot{p}", name=f"ot{p}")
                for p in range(NPAIR)
            ]

            def emit_proj(p):
                """q/k/v projections for pair p; V masked+transposed+augmented."""
                qkv = []
                sts = {}
                for name, xt in (("q", xt_q), ("k", xt_k), ("v", xt_v)):
                    pair_t = proj.tile([128, S], bf16, tag=f"{name}T",
                                       name=f"{name}T{p}")
                    for g in range(2):
                        c = p + 4 * g
                        st = stage.tile([128, KO, S], bf16, tag="xstage",
                                        bufs=4, name=f"st{p}{name}{g}")
                        nc.sync.dma_start(
                            st[:],
                            xt[:, c * S:(c + 1) * S].rearrange(
                                "(ko ki) s -> ki ko s", ki=128
                            ),
                        )
                        sts[(name, g)] = st
                    w_sb = {"q": wq_sb, "k": wk_sb, "v": wv_sb}[name]
                    for hh in range(2):
                        ps = psS.tile([128, 512], f32, tag="ps",
                                      name=f"psp{p}{name}{hh}")
                        for ko in range(KO):
                            for g in range(2):
                                nc.tensor.matmul(
                                    ps[64 * g:64 * (g + 1), :],
                                    lhsT=w_sb[:, ko, :],
                                    rhs=sts[(name, g)][:, ko,
                                                       512 * hh:512 * (hh + 1)],
                                    start=(ko == 0),
                                    stop=(ko == KO - 1),
                                )
                        if name == "v":
                            # fold the key-mask into V while copying out
                            nc.vector.tensor_tensor(
                                pair_t[:, 512 * hh:512 * (hh + 1)],
                                ps[:, :],
                                kmb_sb[:, 512 * hh:512 * (hh + 1)],
                                mybir.AluOpType.mult,
                            )
                        else:
                            nc.scalar.copy(
                                pair_t[:, 512 * hh:512 * (hh + 1)],
                                ps[:, :],
                            )
                    qkv.append(pair_t)
                qT, kT, vm = qkv

                # V back to natural layout via PE transpose, both halves at
                # once: vnat[:, j, g, 0:64] = V_masked block j for batch
                # (p + 4g); vnat[:, j, g, 64] = km column for block j.
                vnat = proj.tile([128, NBLK, 2, DH + 1], bf16, tag="vnat",
                                 name=f"vnat{p}")
                for j in range(NBLK):
                    pst = psT.tile([128, 128], bf16, tag="pst",
                                   name=f"pst{p}{j}")
                    nc.tensor.transpose(
                        pst[:], vm[:, 128 * j:128 * (j + 1)], ident_bf[:]
                    )
                    nc.vector.tensor_copy(vnat[:, j, :, 0:DH], pst[:])
                nc.gpsimd.tensor_copy(
                    vnat[:, :, :, DH],
                    kmt_sb[:, :, None].to_broadcast((128, NBLK, 2)),
                )
                return qT, kT, vnat

            def emit_attn(p, tiles):
                qT, kT, vnat = tiles
                for g in range(2):
                    gs = slice(64 * g, 64 * (g + 1))
                    po = psOT.tile([DH + 1, S], f32, tag="psot",
                                   name=f"po{p}{g}")
                    # seed: rows 0..63 = 0, row 64 = flag-row indicator
                    for n in range(2):
                        cols = slice(512 * n, 512 * (n + 1))
                        nc.tensor.matmul(
                            po[:, cols],
                            lhsT=unit64[:, :],
                            rhs=flg_sb[:, cols],
                            start=True,
                            stop=False,
                            skip_group_check=True,
                        )
                    for j in range(NBLK):
                        w = S - 128 * j      # panel width (sq = 128j .. S)
                        # scores^T panel j, chunked to <=512-col psum tiles
                        chunks = []
                        if j < 4:
                            chunks = [(0, 512 - 128 * j), (512 - 128 * j, w)]
                        else:
                            chunks = [(0, w)]
                        pt = ptp.tile([128, w], bf16, tag="pt",
                                      name=f"pt{p}{g}{j}")
                        for (lo, hi) in chunks:
                            ss = psS.tile([128, 512], f32, tag="ps",
                                          name=f"ss{p}{g}{j}{lo}")
                            nc.tensor.matmul(
                                ss[:, 0:hi - lo],
                                lhsT=kT[gs, 128 * j:128 * (j + 1)],
                                rhs=qT[gs, 128 * j + lo:128 * j + hi],
                                start=True,
                                stop=True,
                            )
                            nc.scalar.activation(
                                pt[:, lo:hi],
                                ss[:, 0:hi - lo],
                                mybir.ActivationFunctionType.Exp,
                                bias=0.0,
                                scale=1.0,
                            )
                            if lo == 0:
                                # causal zeroing of the diagonal block
                                # (sk > sq entries -> 0 after exp)
                                nc.vector.tensor_tensor(
                                    pt[:, 0:128],
                                    pt[:, 0:128],
                                    tri_sb[:],
                                    mybir.AluOpType.mult,
                                )
                        # PV accumulate: po[:, sq] += vnat_j^T @ pt
                        for (lo, hi) in chunks:
                            # bank 0 (cols 0:512) last written at j=3 chunk 0;
                            # bank 1 (cols 512:1024) last written at j=7
                            last_bank = (j == 3 and lo == 0) or (j == 7)
                            nc.tensor.matmul(
                                po[:, 128 * j + lo:128 * j + hi],
                                lhsT=vnat[:, j, g, :],
                                rhs=pt[:, lo:hi],
                                start=False,
                                stop=bool(last_bank),
                                skip_group_check=True,
                            )
                    # normalization: pull the sum row to SBUF, broadcast it
                    # across 64 partitions via a rank-1 fp32r matmul, take
                    # the reciprocal lane-parallel, multiply into O^T.
                    sr = rowp.tile([1, S], f32r, tag="sr", name=f"sr{p}{g}")
                    nc.scalar.copy(sr[:], po[DH:DH + 1, :])
                    for n in range(2):
                        cols = slice(512 * n, 512 * (n + 1))
                        bc = psS.tile([128, 512], f32, tag="ps",
                                      name=f"bc{p}{g}{n}")
                        nc.tensor.matmul(
                            bc[0:DH, :],
                            lhsT=ones64[:, :],
                            rhs=sr[0:1, cols],
                            start=True,
                            stop=True,
                        )
                        rcpb = rowp.tile([DH, 512], f32, tag="bcs",
                                         name=f"rcpb{p}{g}{n}")
                        nc.vector.reciprocal(rcpb[:], bc[0:DH, :])
                        nc.vector.tensor_tensor(
                            ot_sb[p][gs, cols],
                            po[0:DH, cols],
                            rcpb[:],
                            mybir.AluOpType.mult,
                        )
                    # host-computed fully-masked-row corrections (block 0)
                    nc.vector.tensor_tensor(
                        ot_sb[p][gs, 0:128],
                        ot_sb[p][gs, 0:128],
                        corr_sb[gs, p, :],
                        mybir.AluOpType.add,
                    )

            # ---- software-pipelined emission: proj one pair ahead ----
            tiles = emit_proj(0)
            nxt = emit_proj(1)
            emit_attn(0, tiles)
            tiles, nxt = nxt, emit_proj(2)
            emit_attn(1, tiles)
            tiles, nxt = nxt, emit_proj(3)
            emit_attn(2, tiles)
            emit_attn(3, nxt)

            # ---- final projection + relu + query-mask ----
            for i in range(NBLK):
                ps = psS.tile([128, 512], f32, tag="ps", name=f"psf{i}")
                for p in range(NPAIR):
                    nc.tensor.matmul(
                        ps[:],
                        lhsT=ot_sb[p][:, 128 * i:128 * (i + 1)],
                        rhs=wo_sb[:, p, :],
                        start=(p == 0),
                        stop=(p == NPAIR - 1),
                    )
                o_sb = outp.tile([128, D], f32, tag="osb", name=f"osb{i}")
                nc.scalar.activation(
                    o_sb[:],
                    ps[:],
                    mybir.ActivationFunctionType.Relu,
                    bias=0.0,
                    scale=qm_sb[:, i:i + 1],
                )
                nc.sync.dma_start(out_d[128 * i:128 * (i + 1), :], o_sb[:])

    nc.compile()
    return nc


def _get_nc():
    if "nc" not in _CACHE:
        _CACHE["nc"] = _build()
    return _CACHE["nc"]


def _host_prep(query, key, value, query_mask, key_mask, Wq, Wk, Wv, Wo):
    """Build the 8 per-core input maps (numpy only)."""
    inv = np.float32(1.0) / np.sqrt(np.float32(D))

    import ml_dtypes

    def tfeat(x):  # (B,S,D) -> feature-major (D, B*S), contiguous bf16
        return np.ascontiguousarray(
            x.reshape(B * S, D).astype(np.float32, copy=False).T
        ).astype(ml_dtypes.bfloat16)

    xq, xk, xv = tfeat(query), tfeat(key), tfeat(value)
    kmf = key_mask.astype(np.float32)
    qmf = query_mask.astype(np.float32)
    Wqf = Wq.astype(np.float32, copy=False)
    Wkf = Wk.astype(np.float32, copy=False)
    Wvf = Wv.astype(np.float32, copy=False)
    Wof = Wo.astype(np.float32, copy=False)

    wo_p = np.stack(
        [
            np.concatenate(
                [Wof[p * DH:(p + 1) * DH, :], Wof[(p + 4) * DH:(p + 5) * DH, :]],
                axis=0,
            )
            for p in range(NPAIR)
        ]
    ).astype(ml_dtypes.bfloat16)  # (4, 128, 512)

    # causal 0/1 tile for the transposed diagonal block:
    # visible where sk(row) <= sq(col)
    r = np.arange(128)
    tri01 = (r[:, None] <= r[None, :]).astype(ml_dtypes.bfloat16)

    in_maps = []
    for a in range(H):
        km = kmf[a]
        # flag rows: every visible key masked -> faithful-TF uniform tie case
        cs = np.cumsum(km)
        flg = (cs == 0).astype(np.float32)           # (S,)
        corrT = np.zeros((128, NPAIR, 128), np.float32)
        flag_rows = np.nonzero(flg)[0]
        if flag_rows.size:
            assert flag_rows.max() < 128, "flag rows beyond block 0"
            wv_a = Wvf[:, a * DH:(a + 1) * DH]       # (512, 64)
            tail_cnt = km.sum()                      # km[0:sq+1] is all 0
            for p in range(NPAIR):
                for g in range(2):
                    c = p + 4 * g
                    vfull = value[c].astype(np.float32)      # (S, 512)
                    mtot = (km[:, None] * vfull).sum(axis=0)  # (512,)
                    pre = np.zeros(512, np.float32)
                    for sq in flag_rows:
                        pre = vfull[:sq + 1].sum(axis=0)
                        count = (sq + 1) + tail_cnt
                        corrT[64 * g:64 * (g + 1), p, sq] = (
                            (pre + mtot) @ wv_a
                        ) / np.float32(count)
        kmt = km.reshape(NBLK, 128).T                # [r, j] = km[128j + r]
        in_maps.append(
            {
                "xt_q": xq,
                "xt_k": xk,
                "xt_v": xv,
                "wq": np.ascontiguousarray(
                    Wqf[:, a * DH:(a + 1) * DH] * inv
                ).astype(ml_dtypes.bfloat16),
                "wk": np.ascontiguousarray(
                    Wkf[:, a * DH:(a + 1) * DH]
                ).astype(ml_dtypes.bfloat16),
                "wv": np.ascontiguousarray(
                    Wvf[:, a * DH:(a + 1) * DH]
                ).astype(ml_dtypes.bfloat16),
                "wo_p": wo_p,
                "kmb": np.ascontiguousarray(
                    np.broadcast_to(km[None, :], (128, S))
                ).astype(ml_dtypes.bfloat16),
                "kmt": np.ascontiguousarray(kmt).astype(ml_dtypes.bfloat16),
                "tri01": tri01,
                "flg": np.ascontiguousarray(flg[None, :]).astype(
                    ml_dtypes.bfloat16
                ),
                "corrT": corrT,
                "qmask": np.ascontiguousarray(
                    qmf[a].reshape(NBLK, 128).T
                ),  # [p, blk] = qm[a, 128*blk + p]
            }
        )
    return in_maps


def kernel(**inputs) -> np.ndarray:
    from concourse.bass_utils import run_bass_kernel_spmd

    nc = _get_nc()
    in_maps = _host_prep(
        np.asarray(inputs["query"]),
        np.asarray(inputs["key"]),
        np.asarray(inputs["value"]),
        np.asarray(inputs["query_mask"]),
        np.asarray(inputs["key_mask"]),
        np.asarray(inputs["Wq"]),
        np.asarray(inputs["Wk"]),
        np.asarray(inputs["Wv"]),
        np.asarray(inputs["Wo"]),
    )
    res = run_bass_kernel_spmd(nc, in_maps, core_ids=list(range(H)), **RUN_KWARGS)
    global LAST_RESULT
    LAST_RESULT = res
    return np.stack([res.results[a]["out"] for a in range(H)])


# revision 23
# speedup vs baseline: 3.0925x; 1.1297x over previous
"""Trainium2 Bass kernel for nn_MultiHeadAttention_61778809586301.

Head-sharded across 8 NeuronCores: core `a` computes output row-group `a`
(which, per the reference's faithful-TF recombination, is attention head `a`
across all 8 batches, concatenated batch-major along channels, then Wo+relu).

v2 redesign vs v1:
  - scores computed TRANSPOSED (S^T = K^T blocks vs Q panels) so the P^T
    needed by the PV matmul comes straight out of exp() — zero DMA
    transposes of P (v1 spent 400+us in DMA_TRANSPOSE).
  - no max-subtraction softmax: scores are ~N(0, 1/8) so exp() is safe;
    causal-masked entries are min-clipped to -1e9 (exp -> 0).  Key-mask is
    folded into a zeroed-V (V*km) plus an extra km column, so the PV matmul
    emits [O_unnorm | rowsum] in one accumulation; normalization happens
    once on the small O^T (64 x S) instead of on P (S x S).
  - fully-masked-row semantics (the faithful-TF uniform-tie case) handled
    by host-precomputed correction columns added to O^T, plus a flag row
    seeded into the sum via a rank-1 init matmul.
  - V natural layout obtained with PE (tensor-engine) transposes.
"""
import sys

if "/opt/trn_rl_repo" not in sys.path:
    sys.path.insert(0, "/opt/trn_rl_repo")

import numpy as np

B, S, D, H, DH = 8, 1024, 512, 8, 64
NEG = np.float32(1.0e9)
FBIG = np.float32(3.0e38)
NPAIR = 4          # batch pairs (p, p+4)
NBLK = S // 128    # 8 sk/sq blocks of 128
KO = D // 128      # 4 contraction chunks of 128

_CACHE: dict = {}
RUN_KWARGS: dict = {}   # extra kwargs for run_bass_kernel_spmd (e.g. trace)
LAST_RESULT = None      # BassKernelResults of the most recent kernel() call


def _build():
    import concourse.mybir as mybir
    import concourse.tile as tile
    from concourse import bacc
    from concourse.masks import make_identity

    f32 = mybir.dt.float32
    bf16 = mybir.dt.bfloat16
    nc = bacc.Bacc(
        "TRN2",
        target_bir_lowering=False,
        debug=False,
        enable_asserts=False,
        num_devices=H,
    )

    xt_q = nc.dram_tensor("xt_q", [D, B * S], bf16, kind="ExternalInput")
    xt_k = nc.dram_tensor("xt_k", [D, B * S], bf16, kind="ExternalInput")
    xt_v = nc.dram_tensor("xt_v", [D, B * S], bf16, kind="ExternalInput")
    wq_d = nc.dram_tensor("wq", [D, DH], bf16, kind="ExternalInput")
    wk_d = nc.dram_tensor("wk", [D, DH], bf16, kind="ExternalInput")
    wv_d = nc.dram_tensor("wv", [D, DH], bf16, kind="ExternalInput")
    wo_d = nc.dram_tensor("wo_p", [NPAIR, 128, D], bf16, kind="ExternalInput")
    kmb_d = nc.dram_tensor("kmb", [128, S], bf16, kind="ExternalInput")
    kmt_d = nc.dram_tensor("kmt", [128, NBLK], bf16, kind="ExternalInput")
    tri_d = nc.dram_tensor("tri01", [128, 128], bf16, kind="ExternalInput")
    flg_d = nc.dram_tensor("flg", [1, S], bf16, kind="ExternalInput")
    corr_d = nc.dram_tensor("corrT", [128, NPAIR, 128], f32,
                            kind="ExternalInput")
    qm_d = nc.dram_tensor("qmask", [128, NBLK], f32, kind="ExternalInput")
    out_d = nc.dram_tensor("out", [S, D], f32, kind="ExternalOutput")

    with tile.TileContext(nc) as tc:
        with (
            tc.tile_pool(name="fixed", bufs=1) as fixed,
            tc.tile_pool(name="stage", bufs=3) as stage,
            tc.tile_pool(name="proj", bufs=2) as proj,
            tc.tile_pool(name="ptp", bufs=3) as ptp,
            tc.tile_pool(name="rowp", bufs=2) as rowp,
            tc.tile_pool(name="outp", bufs=3) as outp,
            tc.tile_pool(name="psS", bufs=3, space="PSUM") as psS,
            tc.tile_pool(name="psOT", bufs=2, space="PSUM") as psOT,
            tc.tile_pool(name="psT", bufs=1, space="PSUM") as psT,
        ):
            # ---- constants / weights ----
            ident = fixed.tile([128, 128], f32, tag="ident")
            make_identity(nc, ident[:])
            ident_bf = fixed.tile([128, 128], bf16, tag="identbf")
            nc.gpsimd.tensor_copy(ident_bf[:], ident[:])

            wq_sb = fixed.tile([128, KO, DH], bf16, tag="wq")
            wk_sb = fixed.tile([128, KO, DH], bf16, tag="wk")
            wv_sb = fixed.tile([128, KO, DH], bf16, tag="wv")
            nc.sync.dma_start(wq_sb[:], wq_d.rearrange("(ko ki) m -> ki ko m", ki=128))
            nc.sync.dma_start(wk_sb[:], wk_d.rearrange("(ko ki) m -> ki ko m", ki=128))
            nc.sync.dma_start(wv_sb[:], wv_d.rearrange("(ko ki) m -> ki ko m", ki=128))

            wo_sb = fixed.tile([128, NPAIR, D], bf16, tag="wo")
            nc.sync.dma_start(wo_sb[:], wo_d.rearrange("p ki n -> ki p n"))

            qm_sb = fixed.tile([128, NBLK], f32, tag="qm")
            nc.sync.dma_start(qm_sb[:], qm_d[:, :])

            kmb_sb = fixed.tile([128, S], bf16, tag="kmb")
            nc.sync.dma_start(kmb_sb[:], kmb_d[:, :])
            kmt_sb = fixed.tile([128, NBLK], bf16, tag="kmt")
            nc.sync.dma_start(kmt_sb[:], kmt_d[:, :])
            tri_sb = fixed.tile([128, 128], bf16, tag="tri")
            nc.sync.dma_start(tri_sb[:], tri_d[:, :])
            flg_sb = fixed.tile([1, S], bf16, tag="flg")
            nc.sync.dma_start(flg_sb[:], flg_d[:, :])
            corr_sb = fixed.tile([128, NPAIR, 128], f32, tag="corr")
            nc.sync.dma_start(corr_sb[:], corr_d[:, :, :])

            # unit column for the flag-row init (row 64 = 1, others 0) and
            # the ones row for the rcp broadcast
            unit64 = fixed.tile([1, DH + 1], bf16, tag="unit64")
            nc.vector.memset(unit64[:], 0.0)
            nc.vector.memset(unit64[0:1, DH:DH + 1], 1.0)
            f32r = mybir.dt.float32r
            ones64f = fixed.tile([1, DH], f32, tag="ones64f")
            nc.vector.memset(ones64f[:], 1.0)
            ones64 = fixed.tile([1, DH], f32r, tag="ones64")
            nc.scalar.copy(ones64[:], ones64f[:])

            # persistent attention outputs, transposed: [dh of c | dh of c+4] x S
            ot_sb = [
                fixed.tile([128, S], bf16, tag=f"ot{p}", name=f"ot{p}")
                for p in range(NPAIR)
            ]

            def emit_proj(p):
                """q/k/v projections for pair p; V masked+transposed+augmented."""
                qkv = []
                sts = {}
                for name, xt in (("q", xt_q), ("k", xt_k), ("v", xt_v)):
                    pair_t = proj.tile([128, S], bf16, tag=f"{name}T",
                                       name=f"{name}T{p}")
                    for g in range(2):
                        c = p + 4 * g
                        st = stage.tile([128, KO, S], bf16, tag="xstage",
                                        bufs=4, name=f"st{p}{name}{g}")
                        nc.sync.dma_start(
                            st[:],
                            xt[:, c * S:(c + 1) * S].rearrange(
                                "(ko ki) s -> ki ko s", ki=128
                            ),
                        )
                        sts[(name, g)] = st
                    w_sb = {"q": wq_sb, "k": wk_sb, "v": wv_sb}[name]
                    for hh in range(2):
                        ps = psS.tile([128, 512], f32, tag="ps",
                                      name=f"psp{p}{name}{hh}")
                        for ko in range(KO):
                            for g in range(2):
                                nc.tensor.matmul(
                                    ps[64 * g:64 * (g + 1), :],
                                    lhsT=w_sb[:, ko, :],
                                    rhs=sts[(name, g)][:, ko,
                                                       512 * hh:512 * (hh + 1)],
                                    start=(ko == 0),
                                    stop=(ko == KO - 1),
                                )
                        if name == "v":
                            # fold the key-mask into V while copying out
                            nc.vector.tensor_tensor(
                                pair_t[:, 512 * hh:512 * (hh + 1)],
                                ps[:, :],
                                kmb_sb[:, 512 * hh:512 * (hh + 1)],
                                mybir.AluOpType.mult,
                            )
                        else:
                            nc.scalar.copy(
                                pair_t[:, 512 * hh:512 * (hh + 1)],
                                ps[:, :],
                            )
                    qkv.append(pair_t)
                qT, kT, vm = qkv

                # V back to natural layout via PE transpose, both halves at
                # once: vnat[:, j, g, 0:64] = V_masked block j for batch
                # (p + 4g); vnat[:, j, g, 64] = km column for block j.
                vnat = proj.tile([128, NBLK, 2, DH + 1], bf16, tag="vnat",
                                 name=f"vnat{p}")
                for j in range(NBLK):
                    pst = psT.tile([128, 128], bf16, tag="pst",
                                   name=f"pst{p}{j}")
                    nc.tensor.transpose(
                        pst[:], vm[:, 128 * j:128 * (j + 1)], ident_bf[:]
                    )
                    nc.vector.tensor_copy(vnat[:, j, :, 0:DH], pst[:])
                nc.gpsimd.tensor_copy(
                    vnat[:, :, :, DH],
                    kmt_sb[:, :, None].to_broadcast((128, NBLK, 2)),
                )
                return qT, kT, vnat

            def emit_attn(p, tiles):
                qT, kT, vnat = tiles
                for g in range(2):
                    gs = slice(64 * g, 64 * (g + 1))
                    po = psOT.tile([DH + 1, S], f32, tag="psot",
                                   name=f"po{p}{g}")
                    # seed: rows 0..63 = 0, row 64 = flag-row indicator
                    for n in range(2):
                        cols = slice(512 * n, 512 * (n + 1))
                        nc.tensor.matmul(
                            po[:, cols],
                            lhsT=unit64[:, :],
                            rhs=flg_sb[:, cols],
                            start=True,
                            stop=False,
                            skip_group_check=True,
                        )
                    for j in range(NBLK):
                        w = S - 128 * j      # panel width (sq = 128j .. S)
                        # scores^T panel j, chunked to <=512-col psum tiles
                        chunks = []
                        if j < 4:
                            chunks = [(0, 512 - 128 * j), (512 - 128 * j, w)]
                        else:
                            chunks = [(0, w)]
                        pt = ptp.tile([128, w], bf16, tag="pt",
                                      name=f"pt{p}{g}{j}")
                        for (lo, hi) in chunks:
                            ss = psS.tile([128, 512], f32, tag="ps",
                                          name=f"ss{p}{g}{j}{lo}")
                            nc.tensor.matmul(
                                ss[:, 0:hi - lo],
                                lhsT=kT[gs, 128 * j:128 * (j + 1)],
                                rhs=qT[gs, 128 * j + lo:128 * j + hi],
                                start=True,
                                stop=True,
                            )
                            nc.scalar.activation(
                                pt[:, lo:hi],
                                ss[:, 0:hi - lo],
                                mybir.ActivationFunctionType.Exp,
                                bias=0.0,
                                scale=1.0,
                            )
                            if lo == 0:
                                # causal zeroing of the diagonal block
                                # (sk > sq entries -> 0 after exp)
                                nc.vector.tensor_tensor(
                                    pt[:, 0:128],
                                    pt[:, 0:128],
                                    tri_sb[:],
                                    mybir.AluOpType.mult,
                                )
                        # PV accumulate: po[:, sq] += vnat_j^T @ pt
                        for (lo, hi) in chunks:
                            # bank 0 (cols 0:512) last written at j=3 chunk 0;
                            # bank 1 (cols 512:1024) last written at j=7
                            last_bank = (j == 3 and lo == 0) or (j == 7)
                            nc.tensor.matmul(
                                po[:, 128 * j + lo:128 * j + hi],
                                lhsT=vnat[:, j, g, :],
                                rhs=pt[:, lo:hi],
                                start=False,
                                stop=bool(last_bank),
                                skip_group_check=True,
                            )
                    # normalization: pull the sum row to SBUF, broadcast it
                    # across 64 partitions via a rank-1 fp32r matmul, take
                    # the reciprocal lane-parallel, multiply into O^T.
                    sr = rowp.tile([1, S], f32r, tag="sr", name=f"sr{p}{g}")
                    nc.scalar.copy(sr[:], po[DH:DH + 1, :])
                    for n in range(2):
                        cols = slice(512 * n, 512 * (n + 1))
                        bc = psS.tile([128, 512], f32, tag="ps",
                                      name=f"bc{p}{g}{n}")
                        nc.tensor.matmul(
                            bc[0:DH, :],
                            lhsT=ones64[:, :],
                            rhs=sr[0:1, cols],
                            start=True,
                            stop=True,
                        )
                        rcpb = rowp.tile([DH, 512], f32, tag="bcs",
                                         name=f"rcpb{p}{g}{n}")
                        nc.vector.reciprocal_approx_fast(rcpb[:], bc[0:DH, :])
                        nc.vector.tensor_tensor(
                            ot_sb[p][gs, cols],
                            po[0:DH, cols],
                            rcpb[:],
                            mybir.AluOpType.mult,
                        )
                    # host-computed fully-masked-row corrections (block 0)
                    nc.vector.tensor_tensor(
                        ot_sb[p][gs, 0:128],
                        ot_sb[p][gs, 0:128],
                        corr_sb[gs, p, :],
                        mybir.AluOpType.add,
                    )

            # ---- software-pipelined emission: proj one pair ahead ----
            tiles = emit_proj(0)
            nxt = emit_proj(1)
            emit_attn(0, tiles)
            tiles, nxt = nxt, emit_proj(2)
            emit_attn(1, tiles)
            tiles, nxt = nxt, emit_proj(3)
            emit_attn(2, tiles)
            emit_attn(3, nxt)

            # ---- final projection + relu + query-mask ----
            for i in range(NBLK):
                ps = psS.tile([128, 512], f32, tag="ps", name=f"psf{i}")
                for p in range(NPAIR):
                    nc.tensor.matmul(
                        ps[:],
                        lhsT=ot_sb[p][:, 128 * i:128 * (i + 1)],
                        rhs=wo_sb[:, p, :],
                        start=(p == 0),
                        stop=(p == NPAIR - 1),
                    )
                o_sb = outp.tile([128, D], f32, tag="osb", name=f"osb{i}")
                nc.scalar.activation(
                    o_sb[:],
                    ps[:],
                    mybir.ActivationFunctionType.Relu,
                    bias=0.0,
                    scale=qm_sb[:, i:i + 1],
                )
                nc.sync.dma_start(out_d[128 * i:128 * (i + 1), :], o_sb[:])

    nc.compile()
    return nc


def _get_nc():
    if "nc" not in _CACHE:
        _CACHE["nc"] = _build()
    return _CACHE["nc"]


def _host_prep(query, key, value, query_mask, key_mask, Wq, Wk, Wv, Wo):
    """Build the 8 per-core input maps (numpy only)."""
    inv = np.float32(1.0) / np.sqrt(np.float32(D))

    import ml_dtypes

    def tfeat(x):  # (B,S,D) -> feature-major (D, B*S), contiguous bf16
        return np.ascontiguousarray(
            x.reshape(B * S, D).astype(np.float32, copy=False).T
        ).astype(ml_dtypes.bfloat16)

    xq, xk, xv = tfeat(query), tfeat(key), tfeat(value)
    kmf = key_mask.astype(np.float32)
    qmf = query_mask.astype(np.float32)
    Wqf = Wq.astype(np.float32, copy=False)
    Wkf = Wk.astype(np.float32, copy=False)
    Wvf = Wv.astype(np.float32, copy=False)
    Wof = Wo.astype(np.float32, copy=False)

    wo_p = np.stack(
        [
            np.concatenate(
                [Wof[p * DH:(p + 1) * DH, :], Wof[(p + 4) * DH:(p + 5) * DH, :]],
                axis=0,
            )
            for p in range(NPAIR)
        ]
    ).astype(ml_dtypes.bfloat16)  # (4, 128, 512)

    # causal 0/1 tile for the transposed diagonal block:
    # visible where sk(row) <= sq(col)
    r = np.arange(128)
    tri01 = (r[:, None] <= r[None, :]).astype(ml_dtypes.bfloat16)

    in_maps = []
    for a in range(H):
        km = kmf[a]
        # flag rows: every visible key masked -> faithful-TF uniform tie case
        cs = np.cumsum(km)
        flg = (cs == 0).astype(np.float32)           # (S,)
        corrT = np.zeros((128, NPAIR, 128), np.float32)
        flag_rows = np.nonzero(flg)[0]
        if flag_rows.size:
            assert flag_rows.max() < 128, "flag rows beyond block 0"
            wv_a = Wvf[:, a * DH:(a + 1) * DH]       # (512, 64)
            tail_cnt = km.sum()                      # km[0:sq+1] is all 0
            for p in range(NPAIR):
                for g in range(2):
                    c = p + 4 * g
                    vfull = value[c].astype(np.float32)      # (S, 512)
                    mtot = (km[:, None] * vfull).sum(axis=0)  # (512,)
                    pre = np.zeros(512, np.float32)
                    for sq in flag_rows:
                        pre = vfull[:sq + 1].sum(axis=0)
                        count = (sq + 1) + tail_cnt
                        corrT[64 * g:64 * (g + 1), p, sq] = (
                            (pre + mtot) @ wv_a
                        ) / np.float32(count)
        kmt = km.reshape(NBLK, 128).T                # [r, j] = km[128j + r]
        in_maps.append(
            {
                "xt_q": xq,
                "xt_k": xk,
                "xt_v": xv,
                "wq": np.ascontiguousarray(
                    Wqf[:, a * DH:(a + 1) * DH] * inv
                ).astype(ml_dtypes.bfloat16),
                "wk": np.ascontiguousarray(
                    Wkf[:, a * DH:(a + 1) * DH]
                ).astype(ml_dtypes.bfloat16),
                "wv": np.ascontiguousarray(
                    Wvf[:, a * DH:(a + 1) * DH]
                ).astype(ml_dtypes.bfloat16),
                "wo_p": wo_p,
                "kmb": np.ascontiguousarray(
                    np.broadcast_to(km[None, :], (128, S))
                ).astype(ml_dtypes.bfloat16),
                "kmt": np.ascontiguousarray(kmt).astype(ml_dtypes.bfloat16),
                "tri01": tri01,
                "flg": np.ascontiguousarray(flg[None, :]).astype(
                    ml_dtypes.bfloat16
                ),
                "corrT": corrT,
                "qmask": np.ascontiguousarray(
                    qmf[a].reshape(NBLK, 128).T
                ),  # [p, blk] = qm[a, 128*blk + p]
            }
        )
    return in_maps


def kernel(**inputs) -> np.ndarray:
    from concourse.bass_utils import run_bass_kernel_spmd

    nc = _get_nc()
    in_maps = _host_prep(
        np.asarray(inputs["query"]),
        np.asarray(inputs["key"]),
        np.asarray(inputs["value"]),
        np.asarray(inputs["query_mask"]),
        np.asarray(inputs["key_mask"]),
        np.asarray(inputs["Wq"]),
        np.asarray(inputs["Wk"]),
        np.asarray(inputs["Wv"]),
        np.asarray(inputs["Wo"]),
    )
    res = run_bass_kernel_spmd(nc, in_maps, core_ids=list(range(H)), **RUN_KWARGS)
    global LAST_RESULT
    LAST_RESULT = res
    return np.stack([res.results[a]["out"] for a in range(H)])
